# revision 29
# baseline (speedup 1.0000x reference)
"""Trainium2 Bass kernel for batched multi-head attention (v5).

Structure (per core, 2 batch elements, no collectives):
- x -> xT[dim,tok] via bf16 PE transposes; x arrives in per-token-tile
  batched DMAs (one issue per 128-token tile).
- QK^T projection in transposed layout (bf16) with batched weight DMAs
  (one 3D-AP DMA per feature tile instead of 8); V projection natural.
  V is stored as [128, 8, 17*64]: 16 head blocks plus ONE shared ones
  block; the AV lhsT is a strided AP [V_h | ones], so the AV matmul
  [V|1]^T @ P^T leaves AO^T in rows 0:64 and the softmax denominator
  replicated in rows 64:128 of PSUM.
- Scores S^T per head-pair emitted in row-group-alternating order
  (heads 2i/2i+1 live in partition halves -> PE 64x128 tiles T0/T8 can
  stream concurrently); exp on ScalarE PSUM->bf16.
- Normalize via the custom-DVE reciprocal_approx_fast straight off the
  AV PSUM denominator rows (no ScalarE Ln/Exp chain, no SBUF staging).
- Scheduling: a dependency-aware filler queue (selective ensure; loads
  split from matmul chains so DMA latency never blocks the PE queue)
  paced by estimated PE cost, spreading projection work into the
  ScalarE-paced attention loops of both batches; batch-0's
  out-projection fills batch-1's attention tail.  A short warm-up
  matmul burst at t=0 lifts the PE HAM clock gate before real work.
"""

import numpy as np
from collections import deque

_CACHE = {}

B_PER_CORE = 2
N = 1024
DIM = 1024
HEADS = 16
DH = 64
SCALE = DH ** -0.5
N_CORES = 8


def _build_nc():
    import concourse.bass as bass
    from concourse import bacc, mybir, tile
    from concourse.masks import make_identity
    from contextlib import ExitStack

    f32 = mybir.dt.float32
    bf16 = mybir.dt.bfloat16
    Exp = mybir.ActivationFunctionType.Exp
    OpAdd = mybir.AluOpType.add
    OpMult = mybir.AluOpType.mult

    nc = bacc.Bacc(None, target_bir_lowering=False)

    x_e = nc.declare_dram_parameter("x", [B_PER_CORE, N, DIM], f32, isOutput=False)
    wq_e = nc.declare_dram_parameter("w_qkv", [DIM, 3 * DIM], f32, isOutput=False)
    bq_e = nc.declare_dram_parameter("b_qkv", [3 * DIM], f32, isOutput=False)
    wo_e = nc.declare_dram_parameter("w_out", [DIM, DIM], f32, isOutput=False)
    bo_e = nc.declare_dram_parameter("b_out", [DIM], f32, isOutput=False)
    out_e = nc.declare_dram_parameter("out", [B_PER_CORE, N, DIM], f32, isOutput=True)

    with tile.TileContext(nc) as tc, ExitStack() as top:
        singles = top.enter_context(tc.tile_pool(name="singles", bufs=1))
        xtp = top.enter_context(tc.tile_pool(name="xtp", bufs=1))
        qktp = top.enter_context(tc.tile_pool(name="qktp", bufs=1))
        aotp = top.enter_context(tc.tile_pool(name="aotp", bufs=1))
        xip = top.enter_context(tc.tile_pool(name="xip", bufs=3))
        wqkp = top.enter_context(tc.tile_pool(name="wqkp", bufs=3))
        wres = top.enter_context(tc.tile_pool(name="wres", bufs=1))
        oop = top.enter_context(tc.tile_pool(name="oop", bufs=2))
        ptp = top.enter_context(tc.tile_pool(name="ptp", bufs=16))
        rlbp = top.enter_context(tc.tile_pool(name="rlbp", bufs=1))
        # PSUM: pp 2 + st 2x2 + av 2 = 8 banks
        pspp = top.enter_context(tc.tile_pool(name="pspp", bufs=2, space="PSUM"))
        psstp = top.enter_context(tc.tile_pool(name="psstp", bufs=2, space="PSUM"))
        psavp = top.enter_context(tc.tile_pool(name="psavp", bufs=2, space="PSUM"))

        ident = singles.tile([128, 128], f32)
        make_identity(nc, ident)
        ident_bf = singles.tile([128, 128], bf16)
        nc.vector.tensor_copy(out=ident_bf, in_=ident)

        # per-partition bias for the QK^T projection: [feat(128), ftile(16)];
        # pb also hosts the HAM warm-up burst (region 128:256) -- ~36 dense
        # matmuls lift the PE clock gate to 8/8 before real work arrives
        bqk_sb = singles.tile([128, 16], f32)
        tmpb = oop.tile([128, 512], f32, tag="oo", name="tmpb")
        nc.vector.memset(tmpb[:, 0:128], 0.0)
        nc.sync.dma_start(
            out=tmpb[0:16, 0:128],
            in_=bq_e[0 : 2 * DIM].rearrange("(j p) -> j p", j=16),
        )
        pb = pspp.tile([128, 512], f32, tag="pp", name="pb_init")
        for wi in range(36):
            nc.tensor.matmul(
                pb[:, 128:256], lhsT=ident_bf, rhs=ident_bf,
                start=True, stop=True,
            )
        nc.tensor.transpose(pb[:, 0:128], tmpb[:, 0:128], ident)
        nc.vector.tensor_copy(out=bqk_sb, in_=pb[:, 0:16])

        # V natural layout with a 64-wide ones block per (ktile, head), ones
        # FIRST: [1 | V_h] puts the softmax denominator in AV-PSUM rows 0:64
        # (reciprocal_approx_fast requires partition-offset-0 input) and AO^T
        # in rows 64:128
        Vb = singles.tile([128, 8, HEADS, 128], bf16, name="Vb")

        def vb_lhsT(kt, h):
            return Vb[:, kt, h, :]

        # ---------- weight / input loaders (batched DMAs) ----------

        wq_res = {}

        def load_wq(b, ft):
            stg = wqkp.tile(
                [128, 8, 128], f32, tag="wqks", name=f"wqks{b}_{ft}", bufs=2
            )
            nc.gpsimd.dma_start(
                out=stg,
                in_=wq_e[:, ft * 128 : (ft + 1) * 128].rearrange(
                    "(dt p) c -> p dt c", dt=8
                ),
            )
            wt = wqkp.tile([128, 8, 128], bf16, tag="wqk", name=f"wqk{b}_{ft}", bufs=3)
            nc.vector.tensor_copy(out=wt, in_=stg)
            wq_res[(b, ft)] = wt

        wv_res = {}
        wo_res = {}

        def load_w2(kind, tcx):
            res = wv_res if kind == "wv" else wo_res
            wt = wres.tile([128, 8, 512], bf16, tag=f"{kind}{tcx}", name=f"{kind}{tcx}")
            for q in range(4):
                c0 = tcx * 512 + q * 128
                if kind == "wv":
                    src = wq_e[:, 2 * DIM + c0 : 2 * DIM + c0 + 128]
                else:
                    src = wo_e[:, c0 : c0 + 128]
                stg = wqkp.tile(
                    [128, 8, 128], f32, tag="wqks", name=f"{kind}s{tcx}_{q}", bufs=2
                )
                nc.gpsimd.dma_start(
                    out=stg, in_=src.rearrange("(dt p) c -> p dt c", dt=8)
                )
                nc.vector.tensor_copy(
                    out=wt[:, :, q * 128 : (q + 1) * 128], in_=stg
                )
            res[tcx] = wt

        # b_v / b_out broadcast along partitions (bias along the free dim)
        bv_bc = singles.tile([128, DIM], bf16)
        bo_bc = singles.tile([128, DIM], bf16)

        def load_biases():
            for bi, (dst, src) in enumerate(
                ((bv_bc, bq_e[2 * DIM : 3 * DIM]), (bo_bc, bo_e[:]))
            ):
                for h in range(2):
                    seg = src[h * 512 : (h + 1) * 512]
                    stg = oop.tile([128, 512], f32, tag="oo", name=f"bstg{bi}_{h}")
                    nc.gpsimd.dma_start(
                        out=stg,
                        in_=bass.AP(
                            tensor=seg.tensor, offset=seg.offset,
                            ap=[[0, 128], *seg.ap],
                        ),
                    )
                    nc.vector.tensor_copy(out=dst[:, h * 512 : (h + 1) * 512], in_=stg)

        # per-token-tile x loads: one DMA + one cast per 128-token tile
        xin_res = {}

        def load_x(b, tt, eng=None):
            eng = eng if eng is not None else nc.sync
            halves = []
            for dg in range(2):
                xin = xip.tile(
                    [128, 512], f32, tag="xin", name=f"xin{b}_{tt}_{dg}", bufs=2
                )
                eng.dma_start(
                    out=xin,
                    in_=x_e[b, tt * 128 : (tt + 1) * 128, dg * 512 : (dg + 1) * 512],
                )
                xinb = xip.tile(
                    [128, 512], bf16, tag="xinb", name=f"xinb{b}_{tt}_{dg}", bufs=2
                )
                nc.vector.tensor_copy(out=xinb, in_=xin)
                halves.append(xinb)
            xin_res[(b, tt)] = halves

        # ---------- chain builders ----------

        def alloc_batch_tiles(b):
            xT = xtp.tile([128, 8, N], bf16, tag="xt", name=f"xT{b}")
            QKTt = [
                qktp.tile([128, N], bf16, tag=f"qkt{ft}", name=f"qkt{b}_{ft}")
                for ft in range(16)
            ]
            AOT = aotp.tile([128, 8, N], bf16, tag=f"aot{b}", name=f"aot{b}")
            return {"xT": xT, "QKTt": QKTt, "AOT": AOT}

        def make_ph1_mm(bt, b, tt):
            def emit():
                halves = xin_res.pop((b, tt))
                for dg in range(2):
                    ps = pspp.tile([128, 512], bf16, tag="pp", name=f"pst{b}_{tt}_{dg}")
                    for j in range(4):
                        nc.tensor.transpose(
                            ps[:, j * 128 : (j + 1) * 128],
                            halves[dg][:, j * 128 : (j + 1) * 128],
                            ident_bf,
                        )
                    nc.vector.tensor_copy(
                        out=bt["xT"][
                            :, dg * 4 : (dg + 1) * 4, tt * 128 : (tt + 1) * 128
                        ],
                        in_=ps.rearrange("p (j c) -> p j c", j=4),
                    )
            return emit

        def make_qk_mm(bt, b, ft, tcxs=(0, 1)):
            def emit():
                xT, QKTt = bt["xT"], bt["QKTt"]
                wt = wq_res[(b, ft)]
                for tcx in tcxs:
                    pss = pspp.tile(
                        [128, 512], f32, tag="pp", name=f"psq{b}_{ft}_{tcx}"
                    )
                    for dt in range(8):
                        nc.tensor.matmul(
                            pss,
                            lhsT=wt[:, dt, :],
                            rhs=xT[:, dt, tcx * 512 : (tcx + 1) * 512],
                            start=(dt == 0),
                            stop=(dt == 7),
                        )
                    nc.vector.tensor_scalar_add(
                        out=QKTt[ft][:, tcx * 512 : (tcx + 1) * 512],
                        in0=pss,
                        scalar1=bqk_sb[:, ft : ft + 1],
                    )
            return emit

        def make_vproj(bt, b, tcx, mt):
            def emit():
                xT = bt["xT"]
                wvt = wv_res[tcx]
                psv = pspp.tile([128, 512], f32, tag="pp", name=f"psv{b}_{tcx}_{mt}")
                for dt in range(8):
                    nc.tensor.matmul(
                        psv,
                        lhsT=xT[:, dt, mt * 128 : (mt + 1) * 128],
                        rhs=wvt[:, dt, :],
                        start=(dt == 0),
                        stop=(dt == 7),
                    )
                nc.vector.tensor_tensor(
                    out=Vb[:, mt, tcx * 8 : (tcx + 1) * 8, DH:128],
                    in0=psv.rearrange("p (h d) -> p h d", h=8),
                    in1=bv_bc[:, tcx * 512 : (tcx + 1) * 512].rearrange(
                        "p (h d) -> p h d", h=8
                    ),
                    op=OpAdd,
                )
            return emit

        def make_outproj(bt, b, tcx, mt):
            def emit():
                AOT = bt["AOT"]
                wot = wo_res[tcx]
                pso = pspp.tile([128, 512], f32, tag="pp", name=f"pso{b}_{tcx}_{mt}")
                for kt in range(8):
                    nc.tensor.matmul(
                        pso,
                        lhsT=AOT[:, kt, mt * 128 : (mt + 1) * 128],
                        rhs=wot[:, kt, :],
                        start=(kt == 0),
                        stop=(kt == 7),
                    )
                oo = oop.tile([128, 512], f32, tag="oo", name=f"oo{b}_{tcx}_{mt}")
                nc.vector.tensor_tensor(
                    out=oo,
                    in0=pso,
                    in1=bo_bc[:, tcx * 512 : (tcx + 1) * 512],
                    op=OpAdd,
                )
                nc.sync.dma_start(
                    out=out_e[b, mt * 128 : (mt + 1) * 128, tcx * 512 : (tcx + 1) * 512],
                    in_=oo,
                )
            return emit

        # ---------- dependency-aware filler queue ----------

        entries = deque()
        done = set()

        def enq(key, fn, cost, deps=()):
            entries.append({"k": key, "fn": fn, "c": cost, "d": tuple(deps)})

        def _run(e):
            for d in e["d"]:
                ensure(d)
            e["fn"]()
            done.add(e["k"])

        def ensure(*keys):
            for k in keys:
                if k in done:
                    continue
                e = next((e for e in entries if e["k"] == k), None)
                assert e is not None, f"chain {k} not found"
                entries.remove(e)
                _run(e)

        _acc = [0.0]

        def pop_some(slots_left):
            if not entries:
                _acc[0] = 0.0
                return
            total = sum(e["c"] for e in entries)
            _acc[0] = min(_acc[0] + total / max(slots_left, 1), 6000.0)
            while entries and _acc[0] >= entries[0]["c"] * 0.5:
                e = entries.popleft()
                _acc[0] -= e["c"]
                _run(e)

        # ---------- attention ----------

        def emit_attention(bt, b, prefill):
            QKTt, AOT = bt["QKTt"], bt["AOT"]
            for hp in range(8):
                prefill(hp)
                fq, fk = hp, 8 + hp
                ensure(("qk", b, fq), ("qk", b, fk))
                pts = [[], []]
                for kt in range(8):
                    sts = [
                        psstp.tile([128, N], f32, tag="st", name=f"st{b}_{hp}_{kt}_{hi}")
                        for hi in range(2)
                    ]
                    # row-group-alternating emission: heads 2hp/2hp+1 live on
                    # partition halves -> 64x128 PE tiles T0/T8 can overlap
                    for half in range(2):
                        for hi in range(2):
                            koff = hi * 64
                            nc.tensor.matmul(
                                sts[hi][:, half * 512 : (half + 1) * 512],
                                lhsT=QKTt[fk][
                                    koff : koff + 64, kt * 128 : (kt + 1) * 128
                                ],
                                rhs=QKTt[fq][
                                    koff : koff + 64, half * 512 : (half + 1) * 512
                                ],
                                start=True,
                                stop=True,
                            )
                    for hi in range(2):
                        pt = ptp.tile(
                            [128, N], bf16, tag="pt", name=f"pt{b}_{hp}_{kt}_{hi}"
                        )
                        nc.scalar.activation(out=pt, in_=sts[hi], func=Exp, scale=SCALE)
                        pts[hi].append(pt)
                    pop_some((8 - hp) * 10 - kt)

                tcx_need = 0 if hp < 4 else 1
                ensure(*[("v", b, tcx_need, mt) for mt in range(8)])
                for hi in range(2):
                    h = 2 * hp + hi
                    koff = hi * 64
                    avs = []
                    for half in range(2):
                        av = psavp.tile(
                            [128, 512], f32, tag="av", name=f"av{b}_{h}_{half}"
                        )
                        for kt in range(8):
                            nc.tensor.matmul(
                                av,
                                lhsT=vb_lhsT(kt, h),
                                rhs=pts[hi][kt][:, half * 512 : (half + 1) * 512],
                                start=(kt == 0),
                                stop=(kt == 7),
                            )
                        avs.append(av)
                    # normalize straight off PSUM: rows 0:64 hold the
                    # replicated denominator; ~51-ULP reciprocal is plenty
                    for half in range(2):
                        rcp = rlbp.tile(
                            [64, 512], f32, tag="rcp", name=f"rcp{b}_{h}_{half}"
                        )
                        nc.vector.reciprocal_approx_fast(
                            out=rcp, in_=avs[half][0:DH, :]
                        )
                        nc.vector.tensor_tensor(
                            out=AOT[koff : koff + 64, fq, half * 512 : (half + 1) * 512],
                            in0=avs[half][DH:128, :],
                            in1=rcp,
                            op=OpMult,
                        )
                    pop_some((8 - hp) * 10 - 8 - hi)

        # ---------- top-level schedule ----------

        # startup: x(b0) + first weights stream in while the warm-up runs;
        # first QK chains start once half the tokens have transposed.
        # The Vb ones block is replicated by a broadcast DMA from a small
        # ones tile -- a GpSimd memset here would starve DVE via SBUF port
        # contention and stall the whole startup cast chain.
        ones64 = singles.tile([128, 64], bf16)
        nc.vector.memset(ones64, 1.0)
        load_wq(0, 0)
        load_wq(0, 8)
        for tt in range(8):
            load_x(0, tt)
        nc.sync.dma_start(
            out=Vb[:, :, :, 0:DH].rearrange("p a b d -> p (a b) d"),
            in_=bass.AP(
                tensor=ones64.tensor,
                offset=ones64.offset,
                ap=[ones64.ap[0], [0, 8 * HEADS], [1, DH]],
            ),
        )
        load_w2("wv", 0)
        load_biases()

        bt0 = alloc_batch_tiles(0)
        for tt in range(4):
            make_ph1_mm(bt0, 0, tt)()
        make_qk_mm(bt0, 0, 0, (0,))()
        for tt in range(4, 8):
            make_ph1_mm(bt0, 0, tt)()
        make_qk_mm(bt0, 0, 0, (1,))()
        done.add(("qk", 0, 0))
        make_qk_mm(bt0, 0, 8)()
        done.add(("qk", 0, 8))

        for f in (1, 9, 2, 10):
            enq(("wql", 0, f), lambda f=f: load_wq(0, f), 250)
        for mt in range(8):
            enq(("v", 0, 0, mt), make_vproj(bt0, 0, 0, mt), 1800)
        for f in (1, 9):
            enq(("qk", 0, f), make_qk_mm(bt0, 0, f), 3600, deps=[("wql", 0, f)])

        bt1 = alloc_batch_tiles(1)

        def prefill_b0(hp):
            if hp == 0:
                enq(("wvl", 1), lambda: load_w2("wv", 1), 400)
                for mt in range(8):
                    enq(("v", 0, 1, mt), make_vproj(bt0, 0, 1, mt), 1800,
                        deps=[("wvl", 1)])
            # weight loads one head-pair ahead of their matmul chains so the
            # staging DMA+cast never stalls the PE queue
            if hp + 3 < 8:
                for f in (hp + 3, hp + 11):
                    enq(("wql", 0, f), lambda f=f: load_wq(0, f), 250)
            if hp + 2 < 8:
                for f in (hp + 2, hp + 10):
                    enq(("qk", 0, f), make_qk_mm(bt0, 0, f), 3600,
                        deps=[("wql", 0, f)])
            if hp == 4:
                for tt in range(8):
                    enq(("xl", 1, tt), lambda tt=tt: load_x(1, tt), 200)
                for f in (0, 8):
                    enq(("wql", 1, f), lambda f=f: load_wq(1, f), 250)
            if hp == 6:
                # run b0's last QK chains now so the b1 phase-1 drains (which
                # recycle the xT slot) don't stall behind them at hp7
                ensure(("qk", 0, 7), ("qk", 0, 15))
                for tt in range(8):
                    enq(("ph1", 1, tt), make_ph1_mm(bt1, 1, tt), 700,
                        deps=[("xl", 1, tt)])
                for mt in range(8):
                    enq(("v", 1, 0, mt), make_vproj(bt1, 1, 0, mt), 1800,
                        deps=[("ph1", 1, mt)])
            if hp == 7:
                ph1_deps = [("ph1", 1, tt) for tt in range(8)]
                for f in (0, 8):
                    enq(("qk", 1, f), make_qk_mm(bt1, 1, f), 3600,
                        deps=[("wql", 1, f)] + ph1_deps)

        emit_attention(bt0, 0, prefill_b0)

        # batch-0 out-projection + batch-1 remaining projections ride inside
        # batch-1's attention; the queue drains to empty by its last head
        enq(("wol", 0), lambda: load_w2("wo", 0), 400)
        enq(("wol", 1), lambda: load_w2("wo", 1), 400)
        for f in (1, 9, 2, 10):
            enq(("wql", 1, f), lambda f=f: load_wq(1, f), 250)
        for f in (1, 9):
            enq(("qk", 1, f), make_qk_mm(bt1, 1, f), 3600,
                deps=[("wql", 1, f), *[("ph1", 1, tt) for tt in range(8)]])

        def prefill_b1(hp):
            if hp == 0:
                for mt in range(8):
                    enq(("v", 1, 1, mt), make_vproj(bt1, 1, 1, mt), 1800)
                for tcx in range(2):
                    for mt in range(8):
                        enq(("op", 0, tcx, mt), make_outproj(bt0, 0, tcx, mt), 1900,
                            deps=[("wol", tcx)])
            if hp + 3 < 8:
                for f in (hp + 3, hp + 11):
                    enq(("wql", 1, f), lambda f=f: load_wq(1, f), 250)
            if hp + 2 < 8:
                for f in (hp + 2, hp + 10):
                    enq(("qk", 1, f), make_qk_mm(bt1, 1, f), 3600,
                        deps=[("wql", 1, f)])

        emit_attention(bt1, 1, prefill_b1)

        while entries:
            e = entries.popleft()
            _run(e)
        for tcx in range(2):
            for mt in range(8):
                make_outproj(bt1, 1, tcx, mt)()

    return nc


def get_nc():
    if "nc" not in _CACHE:
        nc = _build_nc()
        nc.finalize()
        _CACHE["nc"] = nc
    return _CACHE["nc"]


def make_in_maps(inputs):
    x = np.ascontiguousarray(np.asarray(inputs["x"], dtype=np.float32))
    w_qkv = np.ascontiguousarray(np.asarray(inputs["w_qkv"], dtype=np.float32))
    b_qkv = np.ascontiguousarray(np.asarray(inputs["b_qkv"], dtype=np.float32))
    w_out = np.ascontiguousarray(np.asarray(inputs["w_out"], dtype=np.float32))
    b_out = np.ascontiguousarray(np.asarray(inputs["b_out"], dtype=np.float32))
    in_maps = []
    for c in range(N_CORES):
        in_maps.append(
            {
                "x": np.ascontiguousarray(x[c * B_PER_CORE : (c + 1) * B_PER_CORE]),
                "w_qkv": w_qkv,
                "b_qkv": b_qkv,
                "w_out": w_out,
                "b_out": b_out,
            }
        )
    return in_maps


def run(inputs, trace=False, **kw):
    from concourse.bass_utils import run_bass_kernel_spmd

    nc = get_nc()
    in_maps = make_in_maps(inputs)
    res = run_bass_kernel_spmd(
        nc, in_maps, core_ids=list(range(N_CORES)), trace=trace, **kw
    )
    out = np.concatenate([res.results[c]["out"] for c in range(N_CORES)], axis=0)
    return out, res


def kernel(**inputs):
    out, _ = run(inputs, trace=False)
    return out
